# revision 1
# baseline (speedup 1.0000x reference)
"""AdditiveAttentionLayer Trainium2 kernel (v2).

Math: logits[t,s,b] = scores[s,b] (masked s<t) are t-independent, so
softmax-attention collapses to exclusive prefix sums along T:
    context[t] = (sum_{s<t} e^{scores[s]} * x[s]) / (sum_{s<t} e^{scores[s]})

Per-core (batch-sharded, 4 of 32 batches):
  1. XT = PE-transposes of X;  hp_row[t,k] = tanh(X@W) via lhsT=XT chunks,
     rhs=W;  scores col [128,tt] via fused DVE tensor_tensor_reduce with a
     partition-broadcast proj (one pass: mul+reduce).
  2. Column-layout softmax stats: w = exp(scores) [128, 8];
     z = strict-prefix via one masked matmul + carry rank-1 matmul;
     invz = DVE reciprocal on [128, 8] (128 lanes, not 1);
     invz rows via one PE transpose;  izb (partition-bcast of invz) built
     once per batch via K=1 matmuls.
  3. Per 128-token tile tt: prefix P^T[h,t] with one [128x130] masked
     matmul per h-chunk (strict upper + sum col), running carry S added
     via ACT bias, scaled by izb -> PTZ.
  4. out = tanh(X@Wc1.T + ctxZ@Wc2.T): one PSUM group (16 matmuls) per
     (tt, n-half), single tanh.  Rows 0..1 patched to inputs on host.
"""

import sys
from contextlib import ExitStack

import numpy as np

if "/opt/trn_rl_repo" not in sys.path:
    sys.path.insert(0, "/opt/trn_rl_repo")

import concourse.bass as bass
import concourse.mybir as mybir
from concourse.bacc import Bacc
from concourse.bass_utils import run_bass_kernel_spmd
from concourse.masks import make_identity, make_upper_triangular
from concourse.tile import TileContext

T = 1024
B_FULL = 32
NCORES = 8
BB = B_FULL // NCORES  # 4 batches per core
H = 1024
KC = H // 128  # 8 contraction chunks
NT = T // 128  # 8 t-tiles

F32 = mybir.dt.float32
BF16 = mybir.dt.bfloat16
AF = mybir.ActivationFunctionType
ALU = mybir.AluOpType


def emit_pt(nc, tpps, ptc, wx, u8x_sb, S):
    """Within-tile exclusive prefix for one 128-token tile: masked matmul per
    h-chunk, + running carry S (ACT bias on even chunks, DVE add on odd ones
    to halve the serial drain chain), then S += tile totals."""
    for c in range(KC):
        pt_ps = tpps.tile([128, 130], F32, name="tp")
        nc.tensor.matmul(
            pt_ps,
            wx[:, c * 128 : (c + 1) * 128],
            u8x_sb,
            start=True,
            stop=True,
        )
        if c % 2 == 0:
            nc.scalar.activation(
                ptc[:, c, :], pt_ps[:, 0:128], AF.Identity, bias=S[:, c : c + 1]
            )
        else:
            nc.vector.tensor_scalar_add(
                ptc[:, c, :], pt_ps[:, 0:128], S[:, c : c + 1]
            )
        nc.vector.tensor_add(S[:, c : c + 1], S[:, c : c + 1], pt_ps[:, 128:129])


def build():
    nc = Bacc()

    x_d = nc.dram_tensor("inputs", [T, BB, H], F32, kind="ExternalInput")
    w_d = nc.dram_tensor("W", [H, H], F32, kind="ExternalInput")
    p_d = nc.dram_tensor("proj", [H], F32, kind="ExternalInput")
    cw_d = nc.dram_tensor("concat_w", [H, 2 * H], F32, kind="ExternalInput")
    out_d = nc.dram_tensor("out", [T, BB, H], F32, kind="ExternalOutput")

    with ExitStack() as es:
        tc = es.enter_context(TileContext(nc))

        # ---------------- pools ----------------
        cpool = es.enter_context(tc.tile_pool(name="consts", bufs=1))
        wstg = es.enter_context(tc.tile_pool(name="wstg", bufs=4))
        xfp = es.enter_context(tc.tile_pool(name="xf", bufs=1))
        xtp = es.enter_context(tc.tile_pool(name="xt", bufs=2))
        hprp = es.enter_context(tc.tile_pool(name="hpr", bufs=3))
        scrp = es.enter_context(tc.tile_pool(name="scr", bufs=1))
        smp = es.enter_context(tc.tile_pool(name="sm", bufs=2))
        izbp = es.enter_context(tc.tile_pool(name="izb", bufs=2))
        wxp = es.enter_context(tc.tile_pool(name="wx", bufs=8))
        ptcp = es.enter_context(tc.tile_pool(name="ptc", bufs=1))
        ptzp = es.enter_context(tc.tile_pool(name="ptz", bufs=2))
        op = es.enter_context(tc.tile_pool(name="osb", bufs=2))

        # PSUM banks (8 x 2KB): tpps 2 (transposes + pt prefix share the
        # ring), hpps 2 (hp groups / izb / proj bcast), apsp 2 (out groups),
        # zp 1 (z accumulation group - own bank so interleaved start=True
        # matmuls elsewhere can't clear its has_written bits), auxp 1.
        tpps = es.enter_context(tc.tile_pool(name="tpps", bufs=2, space="PSUM"))
        hpps = es.enter_context(tc.tile_pool(name="hpps", bufs=2, space="PSUM"))
        zp = es.enter_context(tc.tile_pool(name="zp", bufs=1, space="PSUM"))
        auxp = es.enter_context(tc.tile_pool(name="auxp", bufs=1, space="PSUM"))
        apsp = es.enter_context(tc.tile_pool(name="apsp", bufs=2, space="PSUM"))

        # ---------------- constants ----------------
        id_sb = cpool.tile([128, 128], F32, name="id_sb")
        make_identity(nc, id_sb)

        mraw = cpool.tile([128, 130], F32, name="mraw")
        nc.gpsimd.memset(mraw, 0.0)
        make_upper_triangular(nc, mraw[:, 0:128], val=1.0, diag=False)
        u8_sb = cpool.tile([128, 128], BF16, name="u8_sb")
        nc.vector.tensor_copy(u8_sb, mraw[:, 0:128])
        # pt mask: strict upper + ones col 128 + zero col 129
        m2raw = cpool.tile([128, 130], F32, name="m2raw")
        nc.gpsimd.memset(m2raw, 0.0)
        make_upper_triangular(nc, m2raw[:, 0:128], val=1.0, diag=False)
        nc.gpsimd.memset(m2raw[:, 128:129], 1.0)
        u8x_sb = cpool.tile([128, 130], BF16, name="u8x_sb")
        nc.vector.tensor_copy(u8x_sb, m2raw)

        ones_f = cpool.tile([128, 130], F32, name="ones_f")
        nc.gpsimd.memset(ones_f, 1.0)
        ones_b = cpool.tile([128, 130], BF16, name="ones_b")
        nc.vector.tensor_copy(ones_b, ones_f)
        onesf_row = ones_f[0:1, 0:128]
        onesb_col = ones_b[:, 128:129]
        onesb_row = ones_b[0:1, 0:128]

        W_sb = cpool.tile([128, KC, H], BF16, name="W_sb")
        WcT = cpool.tile([128, 2 * KC, H], BF16, name="WcT")
        proj_b = cpool.tile([128, H], BF16, name="proj_b")

        # ---------------- input staging (emission order = DMA priority) ----
        # proj first (tiny, unblocks the proj broadcast matmuls), then
        # batch-0 X (fine-grained for low first-tile latency)
        proj_row = cpool.tile([1, H], F32, name="proj_row")
        nc.sync.dma_start(proj_row, p_d.rearrange("(o k) -> o k", o=1))
        xfs = [None] * BB

        def emit_xf_dma(j):
            xf = xfp.tile([128, NT, H], F32, name="xf")
            for tt in range(NT):
                for q in range(4):
                    nc.sync.dma_start(
                        xf[:, tt, q * 256 : (q + 1) * 256],
                        x_d[tt * 128 : (tt + 1) * 128, j, q * 256 : (q + 1) * 256],
                    )
            return xf

        xfs[0] = emit_xf_dma(0)

        # W chunks -> W_sb (bf16), fine-grained for startup latency
        for c in range(KC):
            stg = wstg.tile([128, H], F32, name="stg")
            for q in range(4):
                nc.sync.dma_start(
                    stg[:, q * 256 : (q + 1) * 256],
                    w_d[c * 128 : (c + 1) * 128, q * 256 : (q + 1) * 256],
                )
            nc.vector.tensor_copy(W_sb[:, c, :], stg)

        # proj -> partition-broadcast bf16 [128, H]
        for g in range(2):
            pb_ps = hpps.tile([128, 512], F32, name="hp_ps")
            nc.tensor.matmul(
                pb_ps,
                onesf_row,
                proj_row[0:1, g * 512 : (g + 1) * 512],
                start=True,
                stop=True,
                skip_group_check=True,
            )
            nc.vector.tensor_copy(proj_b[:, g * 512 : (g + 1) * 512], pb_ps)

        # cw staging DMAs (emitted early so queues prefetch; Wc1 half first)
        cw_stg = {}
        for half in range(2):
            for ro in range(8):
                stg = wstg.tile([128, H], F32, name="stg")
                base = half * H
                nc.sync.dma_start(
                    stg[:, 0:512], cw_d[ro * 128 : (ro + 1) * 128, base : base + 512]
                )
                nc.sync.dma_start(
                    stg[:, 512:1024],
                    cw_d[ro * 128 : (ro + 1) * 128, base + 512 : base + 1024],
                )
                cw_stg[(half, ro)] = stg

        def emit_wct_block(half, ro):
            stg = cw_stg[(half, ro)]
            for cg in range(2):
                tp = tpps.tile([128, 4, 128], F32, name="tp")
                for ci in range(4):
                    c = cg * 4 + ci
                    nc.tensor.transpose(
                        tp[:, ci, :], stg[:, c * 128 : (c + 1) * 128], id_sb
                    )
                nc.scalar.activation(
                    WcT[
                        :,
                        half * KC + cg * 4 : half * KC + (cg + 1) * 4,
                        ro * 128 : (ro + 1) * 128,
                    ],
                    tp,
                    AF.Identity,
                )

        # ---------------- per-batch ----------------
        for j in range(BB):
            xf = xfs[j]

            # ---- phase 1: transposes + hp_row + scores + w + wx per tile ----
            XT = xtp.tile([128, KC, T], BF16, name="XT")
            scores_col = smp.tile([128, NT], F32, name="scores_col")
            w_colf = smp.tile([128, NT], F32, name="w_colf")
            w_col8 = smp.tile([128, NT], BF16, name="w_col8")
            S = smp.tile([128, KC], F32, name="S")
            nc.vector.memset(S, 0.0)
            wxs = []
            for tt in range(NT):
                ttsl = slice(tt * 128, (tt + 1) * 128)
                for cg in range(2):
                    tp = tpps.tile([128, 4, 128], F32, name="tp")
                    for ci in range(4):
                        c = cg * 4 + ci
                        nc.tensor.transpose(
                            tp[:, ci, :], xf[:, tt, c * 128 : (c + 1) * 128], id_sb
                        )
                    nc.vector.tensor_copy(XT[:, cg * 4 : (cg + 1) * 4, ttsl], tp)
            for tt in range(NT):
                ttsl = slice(tt * 128, (tt + 1) * 128)
                hpr = hprp.tile([128, H], BF16, name="hpr")
                for kh in range(2):
                    ksl = slice(kh * 512, (kh + 1) * 512)
                    hp_ps = hpps.tile([128, 512], F32, name="hp_ps")
                    for h in range(KC):
                        nc.tensor.matmul(
                            hp_ps,
                            XT[:, h, ttsl],
                            W_sb[:, h, ksl],
                            start=(h == 0),
                            stop=(h == KC - 1),
                        )
                    nc.scalar.activation(hpr[:, ksl], hp_ps, AF.Tanh)
                scratch = scrp.tile([128, H], BF16, name="scratch")
                nc.vector.tensor_mul(scratch, hpr, proj_b)
                nc.vector.tensor_reduce(
                    scores_col[:, tt : tt + 1],
                    scratch,
                    mybir.AxisListType.X,
                    ALU.add,
                )
                nc.scalar.activation(
                    w_colf[:, tt : tt + 1], scores_col[:, tt : tt + 1], AF.Exp
                )
                nc.vector.tensor_copy(
                    w_col8[:, tt : tt + 1], w_colf[:, tt : tt + 1]
                )
                wx = wxp.tile([128, H], BF16, name="wx")
                nc.vector.tensor_scalar_mul(
                    wx, xf[:, tt, :], w_colf[:, tt : tt + 1]
                )
                wxs.append(wx)

            if j == 0:
                for ro in range(8):
                    emit_wct_block(0, ro)

            # ---- phase 2 + tile 0, resequenced so PE stays busy while the
            # scores->z->invz->izb chain runs on scalar/DVE ----
            # prefix matmuls for tile 0 (only need wx)
            ptc0 = ptcp.tile([128, KC, 128], F32, name="ptc")
            emit_pt(nc, tpps, ptc0, wxs[0], u8x_sb, S)
            # first out group X-half
            a_cover = []
            a_ps = apsp.tile([128, 512], F32, name="a_ps")
            for k in range(KC):
                nc.tensor.matmul(
                    a_ps,
                    XT[:, k, 0:128],
                    WcT[:, k, 0:512],
                    start=(k == 0),
                    stop=False,
                    skip_group_check=True,
                )
            a_cover.append(a_ps)

            z_ps = zp.tile([128, NT], F32, name="z_ps")
            nc.tensor.matmul(
                z_ps, u8_sb, w_col8, start=True, stop=False, skip_group_check=True
            )
            tot_ps = auxp.tile([8, 1], F32, name="aux")
            nc.tensor.matmul(
                tot_ps, w_col8, onesb_col, start=True, stop=True, skip_group_check=True
            )
            tot_col8 = smp.tile([8, 1], BF16, name="tot_col8")
            nc.vector.tensor_copy(tot_col8, tot_ps)
            carry_ps = auxp.tile([1, NT], F32, name="aux")
            nc.tensor.matmul(
                carry_ps,
                tot_col8,
                u8_sb[0:8, 0:8],
                start=True,
                stop=True,
                skip_group_check=True,
            )
            carry_sb = smp.tile([1, NT], BF16, name="carry_sb")
            nc.vector.tensor_copy(carry_sb, carry_ps)
            nc.tensor.matmul(
                z_ps, onesb_row, carry_sb, start=False, stop=True, skip_group_check=True
            )
            invz_col = smp.tile([128, NT], F32, name="invz_col")
            nc.vector.reciprocal(invz_col, z_ps)

            # izb: partition-broadcast of invz.  Build the free-dim broadcast
            # (per-partition scalar) then PE-transpose it.
            izb_all = izbp.tile([128, NT, 128], BF16, name="izb_all")
            for g in range(2):
                izb_ps = hpps.tile([128, 4, 128], F32, name="hp_ps")
                for q in range(4):
                    tt = g * 4 + q
                    icb = smp.tile([128, 128], F32, name="icb")
                    nc.vector.tensor_scalar_mul(
                        icb, ones_f[:, 0:128], invz_col[:, tt : tt + 1]
                    )
                    nc.tensor.transpose(izb_ps[:, q, :], icb, id_sb)
                nc.vector.tensor_copy(izb_all[:, g * 4 : (g + 1) * 4, :], izb_ps)

            # second out group X-half for tile 0
            a_ps = apsp.tile([128, 512], F32, name="a_ps")
            for k in range(KC):
                nc.tensor.matmul(
                    a_ps,
                    XT[:, k, 0:128],
                    WcT[:, k, 512:1024],
                    start=(k == 0),
                    stop=False,
                    skip_group_check=True,
                )
            a_cover.append(a_ps)

            if j == 0:
                for ro in range(8):
                    emit_wct_block(1, ro)

            # ---- phases 3+4 per 128-token tile ----
            for tt in range(NT):
                ttsl = slice(tt * 128, (tt + 1) * 128)
                if tt == 0:
                    ptc = ptc0
                else:
                    ptc = ptcp.tile([128, KC, 128], F32, name="ptc")
                    emit_pt(nc, tpps, ptc, wxs[tt], u8x_sb, S)
                ptz = ptzp.tile([128, KC, 128], BF16, name="ptz")
                for ph in range(2):
                    psl = slice(ph * 4, (ph + 1) * 4)
                    nc.vector.tensor_mul(
                        ptz[:, psl, :],
                        ptc[:, psl, :],
                        izb_all[:, tt, :].unsqueeze(1).broadcast_to([128, 4, 128]),
                    )
                outsb = op.tile([128, H], F32, name="outsb")
                for n in range(2):
                    nsl = slice(n * 512, (n + 1) * 512)
                    if tt == 0:
                        a_ps = a_cover[n]
                    else:
                        a_ps = apsp.tile([128, 512], F32, name="a_ps")
                        for k in range(KC):
                            nc.tensor.matmul(
                                a_ps,
                                XT[:, k, ttsl],
                                WcT[:, k, nsl],
                                start=(k == 0),
                                stop=False,
                            )
                    for k in range(KC):
                        nc.tensor.matmul(
                            a_ps,
                            ptz[:, k, :],
                            WcT[:, KC + k, nsl],
                            start=False,
                            stop=(k == KC - 1),
                            skip_group_check=(tt == 0),
                        )
                    nc.scalar.activation(outsb[:, nsl], a_ps, AF.Tanh)
                for q in range(4):
                    nc.sync.dma_start(
                        out_d[ttsl, j, q * 256 : (q + 1) * 256],
                        outsb[:, q * 256 : (q + 1) * 256],
                    )
                if tt == 3 and j + 1 < BB:
                    xfs[j + 1] = emit_xf_dma(j + 1)

    nc.finalize()
    return nc


_NC = None


def _get_nc():
    global _NC
    if _NC is None:
        _NC = build()
    return _NC


def kernel(**inputs):
    x = np.ascontiguousarray(np.asarray(inputs["inputs"], dtype=np.float32))
    W = np.ascontiguousarray(np.asarray(inputs["W"], dtype=np.float32))
    proj = np.ascontiguousarray(np.asarray(inputs["proj"], dtype=np.float32))
    cw = np.ascontiguousarray(np.asarray(inputs["concat_w"], dtype=np.float32))

    nc = _get_nc()
    in_maps = [
        {
            "inputs": np.ascontiguousarray(x[:, i * BB : (i + 1) * BB, :]),
            "W": W,
            "proj": proj,
            "concat_w": cw,
        }
        for i in range(NCORES)
    ]
    res = run_bass_kernel_spmd(nc, in_maps, core_ids=list(range(NCORES)))
    out = np.concatenate([m["out"] for m in res.results], axis=1)
    out[:2] = x[:2]
    return out



# revision 5
# speedup vs baseline: 1.0323x; 1.0323x over previous
"""AdditiveAttentionLayer Trainium2 kernel (v6: mixed fp8-DR / bf16).

Math: logits[t,s,b] = scores[s,b] (masked s<t) are t-independent, so
softmax-attention collapses to exclusive prefix sums along T:
    context[t] = (sum_{s<t} e^{scores[s]} * x[s]) / (sum_{s<t} e^{scores[s]})

Precision split (measured error budget): the out-GEMM X-part dominates the
signal, so it stays bf16; the hp GEMM (softmax washes quantization out) and
the ctx-part (small relative magnitude) run fp8e4 DoubleRow at 2x rate.

Per-core (batch-sharded, 4 of 32 batches):
  1. XTb = bf16(8*X^T) via PE transposes + ACT-copy(scale=8); XT8 =
     fp8(XTb) via gpsimd TT (idle engine); hp = tanh(X@W) via fp8 DR
     (XT8 pairs x W8 pairs, W8 = fp8(16W)), ACT Tanh(scale=1/128);
     scores via fused DVE scalar_tensor_tensor (mul + accum reduce)
     against a partition-broadcast proj.
  2. w = exp(scores) col [128, NT]; z = strict-prefix via masked matmul +
     carry rank-1 matmul; izb = [128, T] broadcast of 4/z
     (transpose-to-psum-row + K=1 matmul broadcast).
  3. Prefix P^T[h,t]: one [128x130] masked matmul per h-chunk, 2 chunks
     share a PSUM bank (start=T/F), running carry S added and izb scale
     applied in ONE fused DVE scalar_tensor_tensor -> ptz8 fp8 (= 4*ctx);
     S updated from the in-mask totals column, one strided add per pair.
  4. out = tanh(X@Wc1.T + ctx@Wc2.T): 8 bf16 matmuls (XTb x WcT1b =
     bf16(8*Wc1^T)) + 4 fp8 DR (ptz8 x WcT8f = fp8(16*Wc2^T)) per
     n-half into one PSUM group; single ACT Tanh(scale=1/64).
  Rows 0..1 patched to inputs on host.
"""

import sys
from contextlib import ExitStack

import numpy as np

if "/opt/trn_rl_repo" not in sys.path:
    sys.path.insert(0, "/opt/trn_rl_repo")

import concourse.bass as bass
import concourse.mybir as mybir
from concourse.bacc import Bacc
from concourse.bass_utils import run_bass_kernel_spmd
from concourse.masks import make_identity, make_upper_triangular
from concourse.tile import TileContext

T = 1024
B_FULL = 32
NCORES = 8
BB = B_FULL // NCORES  # 4 batches per core
H = 1024
KC = H // 128  # 8 contraction chunks
NP = KC // 2  # 4 fp8 DoubleRow chunk-pairs
NT = T // 128  # 8 t-tiles

F32 = mybir.dt.float32
BF16 = mybir.dt.bfloat16
F8 = mybir.dt.float8e4
AF = mybir.ActivationFunctionType
ALU = mybir.AluOpType
DR = mybir.MatmulPerfMode.DoubleRow

SX = 8.0  # X scale (both XTb bf16 and XT8 fp8 carry 8*X)
SW = 16.0  # W / Wc2 fp8 scale
SW1 = 8.0  # Wc1 bf16 scale
S_HP = 1.0 / (SX * SW)  # 1/128, hp psum descale
S_OUT = 1.0 / (SX * SW1)  # 1/64, out psum descale (= 4*16 on ctx side)


def build():
    nc = Bacc()

    x_d = nc.dram_tensor("inputs", [T, BB, H], F32, kind="ExternalInput")
    w_d = nc.dram_tensor("W", [H, H], F32, kind="ExternalInput")
    p_d = nc.dram_tensor("proj", [H], F32, kind="ExternalInput")
    cw_d = nc.dram_tensor("concat_w", [H, 2 * H], F32, kind="ExternalInput")
    out_d = nc.dram_tensor("out", [T, BB, H], F32, kind="ExternalOutput")

    with ExitStack() as es:
        tc = es.enter_context(TileContext(nc))

        # ---------------- pools ----------------
        cpool = es.enter_context(tc.tile_pool(name="consts", bufs=1))
        wstg = es.enter_context(tc.tile_pool(name="wstg", bufs=2))
        xfp = es.enter_context(tc.tile_pool(name="xf", bufs=10))
        xtbp = es.enter_context(tc.tile_pool(name="xtb", bufs=2))
        xt8p = es.enter_context(tc.tile_pool(name="xt8", bufs=2))
        wxp = es.enter_context(tc.tile_pool(name="wx", bufs=16))
        hprp = es.enter_context(tc.tile_pool(name="hpr", bufs=2))
        scrp = es.enter_context(tc.tile_pool(name="scr", bufs=1))
        smp = es.enter_context(tc.tile_pool(name="sm", bufs=2))
        izbp = es.enter_context(tc.tile_pool(name="izb", bufs=2))
        ptz8p = es.enter_context(tc.tile_pool(name="ptz8", bufs=2))
        op = es.enter_context(tc.tile_pool(name="osb", bufs=2))

        # PSUM (8 banks): tpps 2 (X/Wc transposes + izrow), hpps 2 (hp
        # groups + proj/izb broadcasts + tot/carry), ptzp 2 (prefix pairs
        # + z group), apsp 2 (out groups).
        tpps = es.enter_context(tc.tile_pool(name="tpps", bufs=2, space="PSUM"))
        hpps = es.enter_context(tc.tile_pool(name="hpps", bufs=2, space="PSUM"))
        ptzp = es.enter_context(tc.tile_pool(name="ptzp", bufs=2, space="PSUM"))
        apsp = es.enter_context(tc.tile_pool(name="apsp", bufs=2, space="PSUM"))

        # ---------------- constants ----------------
        id_sb = cpool.tile([128, 128], F32, name="id_sb")
        make_identity(nc, id_sb)
        id_b16 = cpool.tile([128, 128], BF16, name="id_b16")
        nc.vector.tensor_copy(id_b16, id_sb)

        mraw = cpool.tile([128, 130], F32, name="mraw")
        nc.gpsimd.memset(mraw, 0.0)
        make_upper_triangular(nc, mraw[:, 0:128], val=1.0, diag=False)
        u8_sb = cpool.tile([128, 128], BF16, name="u8_sb")
        nc.vector.tensor_copy(u8_sb, mraw[:, 0:128])
        # prefix mask: strict upper + ones col 128 + zero col 129
        nc.gpsimd.memset(mraw[:, 128:129], 1.0)
        u8x_sb = cpool.tile([128, 130], BF16, name="u8x_sb")
        nc.vector.tensor_copy(u8x_sb, mraw)

        ones_f = cpool.tile([128, 130], F32, name="ones_f")
        nc.gpsimd.memset(ones_f, 1.0)
        ones_b = cpool.tile([128, 130], BF16, name="ones_b")
        nc.vector.tensor_copy(ones_b, ones_f)
        onesf_row = ones_f[0:1, 0:128]
        onesb_col = ones_b[:, 128:129]
        onesb_row = ones_b[0:1, 0:128]
        ones_128b = ones_b[:, 0:128]

        W8 = cpool.tile([128, KC, H], F8, name="W8")
        WcT1b = cpool.tile([128, KC, H], BF16, name="WcT1b")
        WcT8f = cpool.tile([128, KC, H], F8, name="WcT8f")
        proj_b = cpool.tile([128, H], BF16, name="proj_b")

        # ---------------- input staging (emission order = DMA priority) ----
        proj_row = cpool.tile([1, H], F32, name="proj_row")
        nc.sync.dma_start(proj_row, p_d.rearrange("(o k) -> o k", o=1))
        xfs = [[None] * NT for _ in range(BB)]

        def emit_xf_dma(j, fine):
            for tt in range(NT):
                xf = xfp.tile([128, H], F32, name="xf")
                if fine and tt < 2:
                    for q in range(4):
                        nc.sync.dma_start(
                            xf[:, q * 256 : (q + 1) * 256],
                            x_d[
                                tt * 128 : (tt + 1) * 128, j, q * 256 : (q + 1) * 256
                            ],
                        )
                else:
                    nc.sync.dma_start(xf, x_d[tt * 128 : (tt + 1) * 128, j, :])
                xfs[j][tt] = xf

        emit_xf_dma(0, fine=True)

        # proj -> partition-broadcast bf16 [128, H]
        for g in range(2):
            pb_ps = hpps.tile([128, 512], F32, name="hp_ps")
            nc.tensor.matmul(
                pb_ps,
                onesf_row,
                proj_row[0:1, g * 512 : (g + 1) * 512],
                start=True,
                stop=True,
                skip_group_check=True,
            )
            nc.vector.tensor_copy(proj_b[:, g * 512 : (g + 1) * 512], pb_ps)

        # W chunks -> W8 (fp8, x16)
        for c in range(KC):
            stg = wstg.tile([128, H], F32, name="stg")
            nc.sync.dma_start(stg, w_d[c * 128 : (c + 1) * 128, :])
            nc.vector.tensor_scalar_mul(W8[:, c, :], stg, SW)

        # cw staging DMAs (Wc1 half first)
        cw_stg = {}
        for half in range(2):
            for ro in range(8):
                stg = wstg.tile([128, H], F32, name="stg")
                base = half * H
                nc.sync.dma_start(
                    stg, cw_d[ro * 128 : (ro + 1) * 128, base : base + H]
                )
                cw_stg[(half, ro)] = stg

        def emit_wct_block(half, ro):
            stg = cw_stg[(half, ro)]
            for cg in range(2):
                tp = tpps.tile([128, 4, 128], F32, name="tp")
                for ci in range(4):
                    c = cg * 4 + ci
                    nc.tensor.transpose(
                        tp[:, ci, :], stg[:, c * 128 : (c + 1) * 128], id_sb
                    )
                dst = WcT1b if half == 0 else WcT8f
                nc.scalar.mul(
                    dst[:, cg * 4 : (cg + 1) * 4, ro * 128 : (ro + 1) * 128],
                    tp,
                    SW1 if half == 0 else SW,
                )

        wxs = [[None] * NT for _ in range(BB)]
        sms = [None] * BB

        def emit_phase1(j):
            """Transposes + XTb/XT8 + hp (fp8 DR) + scores + w + wx per tile."""
            XTb = xtbp.tile([128, KC, T], BF16, name="XTb")
            XT8 = xt8p.tile([128, KC, T], F8, name="XT8")
            scores_col = smp.tile([128, NT], F32, name="scores_col")
            w_colf = smp.tile([128, NT], F32, name="w_colf")
            w_col8 = smp.tile([128, NT], BF16, name="w_col8")
            S = smp.tile([128, KC], F32, name="S")
            nc.vector.memset(S, 0.0)
            sms[j] = (XTb, XT8, scores_col, w_colf, w_col8, S)
            for tt in range(NT):
                ttsl = slice(tt * 128, (tt + 1) * 128)
                xf = xfs[j][tt]
                for cg in range(2):
                    cgs = slice(cg * 4, (cg + 1) * 4)
                    tp = tpps.tile([128, 4, 128], F32, name="tp")
                    for ci in range(4):
                        c = cg * 4 + ci
                        nc.tensor.transpose(
                            tp[:, ci, :], xf[:, c * 128 : (c + 1) * 128], id_sb
                        )
                    nc.scalar.mul(XTb[:, cgs, ttsl], tp, SX)
                    nc.gpsimd.tensor_tensor(
                        XT8[:, cgs, ttsl],
                        XTb[:, cgs, ttsl],
                        ones_128b.unsqueeze(1).broadcast_to([128, 4, 128]),
                        ALU.mult,
                    )
                hpr = hprp.tile([128, H], BF16, name="hpr")
                for kh in range(2):
                    ksl = slice(kh * 512, (kh + 1) * 512)
                    hp_ps = hpps.tile([128, 512], F32, name="hp_ps")
                    for p in range(NP):
                        nc.tensor.matmul(
                            hp_ps,
                            XT8[:, 2 * p : 2 * p + 2, ttsl],
                            W8[:, 2 * p : 2 * p + 2, ksl],
                            start=(p == 0),
                            stop=(p == NP - 1),
                            perf_mode=DR,
                        )
                    nc.scalar.activation(hpr[:, ksl], hp_ps, AF.Tanh, scale=S_HP)
                scratch = scrp.tile([128, H], BF16, name="scratch")
                nc.vector.scalar_tensor_tensor(
                    scratch,
                    hpr,
                    1.0,
                    proj_b,
                    ALU.mult,
                    ALU.mult,
                    accum_out=scores_col[:, tt : tt + 1],
                )
                nc.scalar.activation(
                    w_colf[:, tt : tt + 1], scores_col[:, tt : tt + 1], AF.Exp
                )
                nc.vector.tensor_copy(w_col8[:, tt : tt + 1], w_colf[:, tt : tt + 1])
                wx = wxp.tile([128, H], BF16, name="wx")
                nc.vector.tensor_scalar_mul(wx, xf, w_colf[:, tt : tt + 1])
                wxs[j][tt] = wx
                if tt == 3 and j + 1 < BB:
                    emit_xf_dma(j + 1, fine=False)
            if j == 0:
                for ro in range(8):
                    emit_wct_block(0, ro)

        def emit_phase234(j):
            XTb, XT8, scores_col, w_colf, w_col8, S = sms[j]

            # ---- z chain: z = strict-prefix of w (cross-tile carry) ----
            z_ps = ptzp.tile([128, NT], F32, name="pt_ps")
            nc.tensor.matmul(
                z_ps, u8_sb, w_col8, start=True, stop=False, skip_group_check=True
            )
            tot_ps = hpps.tile([8, 1], F32, name="hp_ps")
            nc.tensor.matmul(
                tot_ps, w_col8, onesb_col, start=True, stop=True,
                skip_group_check=True,
            )
            tot_col8 = smp.tile([8, 1], BF16, name="tot_col8")
            nc.vector.tensor_copy(tot_col8, tot_ps)
            carry_ps = hpps.tile([1, NT], F32, name="hp_ps")
            nc.tensor.matmul(
                carry_ps,
                tot_col8,
                u8_sb[0:8, 0:8],
                start=True,
                stop=True,
                skip_group_check=True,
            )
            carry_sb = smp.tile([1, NT], BF16, name="carry_sb")
            nc.vector.tensor_copy(carry_sb, carry_ps)
            nc.tensor.matmul(
                z_ps, onesb_row, carry_sb, start=False, stop=True,
                skip_group_check=True,
            )
            invz_colf = smp.tile([128, NT], F32, name="invz_colf")
            nc.vector.reciprocal(invz_colf, z_ps)
            iz4_col8 = smp.tile([128, NT], BF16, name="iz4_col8")
            nc.vector.tensor_scalar_mul(iz4_col8, invz_colf, 4.0)

            # izb = [128, T] broadcast of 4/z: transpose cols to one psum
            # row, then K=1 matmul broadcast.
            izrow_ps = tpps.tile([1, T], BF16, name="tp")
            for tt in range(NT):
                nc.tensor.matmul(
                    izrow_ps[0:1, tt * 128 : (tt + 1) * 128],
                    iz4_col8[:, tt : tt + 1],
                    id_b16,
                    is_transpose=True,
                    start=(tt == 0),
                    stop=(tt == NT - 1),
                    skip_group_check=True,
                )
            izrow_sb = smp.tile([1, T], BF16, name="izrow_sb")
            nc.scalar.copy(izrow_sb, izrow_ps)
            izb = izbp.tile([128, T], BF16, name="izb")
            for g in range(2):
                izb_ps = hpps.tile([128, 512], F32, name="hp_ps")
                nc.tensor.matmul(
                    izb_ps,
                    onesb_row,
                    izrow_sb[0:1, g * 512 : (g + 1) * 512],
                    start=True,
                    stop=True,
                    skip_group_check=True,
                )
                nc.vector.tensor_copy(izb[:, g * 512 : (g + 1) * 512], izb_ps)

            if j == 0:
                for ro in range(8):
                    emit_wct_block(1, ro)

            # ---- prefix + out GEMM per tile ----
            ptz8 = ptz8p.tile([128, KC, T], F8, name="ptz8")
            for tt in range(NT):
                ttsl = slice(tt * 128, (tt + 1) * 128)
                wx = wxs[j][tt]
                for p in range(NP):
                    pt_ps = ptzp.tile([128, 2, 130], F32, name="pt_ps")
                    for i in range(2):
                        c = 2 * p + i
                        nc.tensor.matmul(
                            pt_ps[:, i, :],
                            wx[:, c * 128 : (c + 1) * 128],
                            u8x_sb,
                            start=(i == 0),
                            stop=(i == 1),
                            skip_group_check=True,
                        )
                    for i in range(2):
                        c = 2 * p + i
                        nc.vector.scalar_tensor_tensor(
                            ptz8[:, c, ttsl],
                            pt_ps[:, i, 0:128],
                            S[:, c : c + 1],
                            izb[:, ttsl],
                            ALU.add,
                            ALU.mult,
                        )
                    nc.vector.tensor_add(
                        S[:, 2 * p : 2 * p + 2], S[:, 2 * p : 2 * p + 2],
                        pt_ps[:, 0:2, 128],
                    )
                outsb = op.tile([128, H], F32, name="outsb")
                for n in range(2):
                    nsl = slice(n * 512, (n + 1) * 512)
                    a_ps = apsp.tile([128, 512], F32, name="a_ps")
                    for k in range(KC):
                        nc.tensor.matmul(
                            a_ps,
                            XTb[:, k, ttsl],
                            WcT1b[:, k, nsl],
                            start=(k == 0),
                            stop=False,
                        )
                    for p in range(NP):
                        nc.tensor.matmul(
                            a_ps,
                            ptz8[:, 2 * p : 2 * p + 2, ttsl],
                            WcT8f[:, 2 * p : 2 * p + 2, nsl],
                            start=False,
                            stop=(p == NP - 1),
                            perf_mode=DR,
                        )
                    nc.scalar.activation(outsb[:, nsl], a_ps, AF.Tanh, scale=S_OUT)
                nc.sync.dma_start(out_d[ttsl, j, :], outsb)

        for j in range(BB):
            emit_phase1(j)
            if j > 0:
                emit_phase234(j - 1)
        emit_phase234(BB - 1)

    nc.finalize()
    return nc


_NC = None


def _get_nc():
    global _NC
    if _NC is None:
        _NC = build()
    return _NC


def kernel(**inputs):
    x = np.ascontiguousarray(np.asarray(inputs["inputs"], dtype=np.float32))
    W = np.ascontiguousarray(np.asarray(inputs["W"], dtype=np.float32))
    proj = np.ascontiguousarray(np.asarray(inputs["proj"], dtype=np.float32))
    cw = np.ascontiguousarray(np.asarray(inputs["concat_w"], dtype=np.float32))

    nc = _get_nc()
    in_maps = [
        {
            "inputs": np.ascontiguousarray(x[:, i * BB : (i + 1) * BB, :]),
            "W": W,
            "proj": proj,
            "concat_w": cw,
        }
        for i in range(NCORES)
    ]
    res = run_bass_kernel_spmd(nc, in_maps, core_ids=list(range(NCORES)))
    out = np.concatenate([m["out"] for m in res.results], axis=1)
    out[:2] = x[:2]
    return out
